# revision 25
# baseline (speedup 1.0000x reference)
"""Trainium2 Bass kernel for nn_NeRFMLPNetwork (StyleGAN-style modulated 1x1-conv MLP).

Network (per layer): s = affine(w_lat); y = conv1x1(x * s); y = y * rsqrt(demod) + b;
out = lrelu(y) * sqrt(2).  8 layers (60->128, then 7x 128->128), B=4, H*W=32768.

Strategy:
  - Data parallel over H*W: each of 8 cores handles 4096 spatial points (all batches).
  - Per (layer, batch) fold modulation s into the weight: Wmod[c,o] = convT[c,o]*s[b,c],
    kept in f32r (full-rate matmul, ~2^-13 precision).  Demod scale d and bias are
    applied in the epilogue: out = prelu(psum*dscale + sqrt2*cb, alpha=0.2), where
    dscale = sqrt(2/(v+eps)) folds in the sqrt(2) lrelu gain.
  - Epilogue split between ScalarE (Prelu activation) and VectorE (custom DVE op
    SCALE_BIAS_LRELU: out = max(z, 0.2z), z = in*s0+s1) so neither engine bottlenecks.
  - Style path (tiny) on device: s via PE matmul over 4 K-chunks, demod sum via PE
    matmul of squared weights, sqrt on ScalarE, reciprocal on VectorE.

Host-side prep is layout only: transposes/reshapes of the small parameter tensors
plus folding the constant sqrt(2) into the conv bias.
"""

import numpy as np

import concourse.bacc as bacc
import concourse.mybir as mybir
import concourse.tile as tile
from concourse.bass_utils import run_bass_kernel_spmd

# ---------------------------------------------------------------------------
# Custom DVE op: out = max(z, z*imm2) with z = in0*s0 + s1   (leaky relu)
# ---------------------------------------------------------------------------
import concourse.dve_ops as dve_ops_mod
from concourse.dve_spec import Spec, Src0, C0, C1, C2, maxx, lower as _dve_lower
from concourse.dve_spec import _has_src1
from concourse.dve_uop import DveOpSpec


def _sbl_ref(in0, in1, s0, s1, imm2):
    z = in0.astype(np.float32) * s0 + s1
    return np.maximum(z, z * imm2)


_z = Src0 * C0 + C1
_SBL_SPEC = Spec(body=maxx(_z, _z * C2), reference=_sbl_ref)
SCALE_BIAS_LRELU = dve_ops_mod.DveOp(
    "SCALE_BIAS_LRELU", _SBL_SPEC, subdim=False, uops_sha={}
)
if "SCALE_BIAS_LRELU" not in dve_ops_mod._SUB_OPCODE_FOR_NAME:
    dve_ops_mod.OPS.append(SCALE_BIAS_LRELU)
    dve_ops_mod.CUSTOM_DVE_SPECS["SCALE_BIAS_LRELU"] = _SBL_SPEC
    dve_ops_mod._SUB_OPCODE_FOR_NAME["SCALE_BIAS_LRELU"] = (
        max(dve_ops_mod._SUB_OPCODE_FOR_NAME.values()) + 1
    )
for _ver in ("v3", "v4"):
    _s = DveOpSpec(
        name="SCALE_BIAS_LRELU",
        opcode=dve_ops_mod.get_dve_sub_opcode("SCALE_BIAS_LRELU"),
        uops=_dve_lower(_SBL_SPEC, ver=_ver),
        rd1_en=_has_src1(_SBL_SPEC),
    )
    SCALE_BIAS_LRELU.uops_sha[_ver] = _s.sha(_ver)

# ---------------------------------------------------------------------------
# Problem constants (hardcoded per spec)
# ---------------------------------------------------------------------------
B, CIN, H, W, HID, WDIM, NB = 4, 60, 64, 512, 128, 512, 8
HWTOT = H * W                    # 32768
N_CORES = 8
SHARD = HWTOT // N_CORES         # 4096 spatial points per core
INV_SQRT_WDIM = float(1.0 / np.sqrt(WDIM))
SQRT2 = float(np.sqrt(2.0))
EPS = 1e-8

F32 = mybir.dt.float32
F32R = mybir.dt.float32r

GROUP = 1024                     # psum group columns (2 banks)
BLKCOLS = 4096                   # columns per processing block
SPLIT = 512                      # epilogue cols on ScalarE (bank-aligned); rest VectorE
NT = GROUP // 512                # matmuls per psum group
EPI_MODE = "split"               # 'split'(group-alternating) | 'splitcol' | 'act' | 'dve' | 'none'
ACT_SHARE = 69                   # of ACT_DEN groups go to ScalarE (rest VectorE)
ACT_DEN = 128
EPI_BAL = "mod"                  # 'greedy' | 'mod'
BLOCK_SPLIT = False              # split first/last batch into halves
SPLIT_LAST = False               # split only the last batch into halves
STORE_Q = "tail-alt"             # 'pool' | 'alt' | 'tail-alt'
X0_Q = "pool"                    # 'sync' | 'pool' (block-0 x via SWDGE, 2 chunks)
W_EARLY = False                  # cTr/affTr[0] + abr/gcbr before the rest

_COMPILED = None


def _build(K=1):
    """Build the program; K>1 unrolls the whole pipeline K times (for timing)."""
    nc = bacc.Bacc("TRN2", target_bir_lowering=False, debug=False,
                   num_devices=N_CORES)

    # x is declared f32r: raw f32 bits DMA directly; the PE rounds on read
    # (verified bit-identical to a DVE f32->f32r rounding copy).
    x_d = nc.dram_tensor("x", [B, CIN, SHARD], F32R, kind="ExternalInput").ap()
    wpT_d = nc.dram_tensor("wpT", [128, 4, NB, B], F32, kind="ExternalInput").ap()
    affT0_d = nc.dram_tensor("affT0", [128, 4, CIN], F32, kind="ExternalInput").ap()
    affTr_d = nc.dram_tensor("affTr", [128, 4, NB - 1, HID], F32, kind="ExternalInput").ap()
    ab0_d = nc.dram_tensor("ab0", [CIN, 1], F32, kind="ExternalInput").ap()
    abr_d = nc.dram_tensor("abr", [HID, NB - 1], F32, kind="ExternalInput").ap()
    cT0_d = nc.dram_tensor("cT0", [CIN, HID], F32, kind="ExternalInput").ap()
    cTr_d = nc.dram_tensor("cTr", [HID, NB - 1, HID], F32, kind="ExternalInput").ap()
    gcb0_d = nc.dram_tensor("gcb0", [HID, 1], F32, kind="ExternalInput").ap()
    gcbr_d = nc.dram_tensor("gcbr", [HID, NB - 1], F32, kind="ExternalInput").ap()
    y_d = nc.dram_tensor("y", [B, HID, SHARD], F32, kind="ExternalOutput").ap()

    COLS = B * SHARD             # 16384 columns resident per core

    with tile.TileContext(nc) as tc:
        with (
            tc.tile_pool(name="big", bufs=4) as big,
            tc.tile_pool(name="wts", bufs=1) as wts,
            tc.tile_pool(name="xst", bufs=4) as xst,
            tc.tile_pool(name="sty", bufs=3) as sty,
            tc.tile_pool(name="wmod", bufs=NB + 1) as wmodp,
            tc.tile_pool(name="dsc", bufs=NB + 1) as dscp,
            tc.tile_pool(name="ps", bufs=4, space="PSUM") as ps,
        ):
            # ---- DMA order: style(0)'s weights first, then block-0 input,
            # then everything else.  The SP sequencer serializes DMA issues
            # at ~650ns each, so x0 must not queue behind the 16 per-layer
            # weight DMAs (that alone put the main-loop start at ~14us). ----
            wpT = wts.tile([128, 4, NB, B], F32, tag="wpT")
            nc.sync.dma_start(wpT[:], wpT_d[:])
            affT0 = wts.tile([128, 4, CIN], F32, tag="affT0")
            nc.sync.dma_start(affT0[:], affT0_d[:])
            ab0 = wts.tile([CIN, 1], F32, tag="ab0")
            nc.sync.dma_start(ab0[:], ab0_d[:])
            cT0 = wts.tile([CIN, HID], F32, tag="cT0")
            nc.sync.dma_start(cT0[:], cT0_d[:])
            x0A = big.tile([128, SHARD], F32R, tag="xbuf")
            x0B = big.tile([128, SHARD], F32R, tag="xbuf")
            if X0_Q == "pool":
                nc.gpsimd.dma_start(x0A[:CIN, :SHARD // 2],
                                    x_d[0, :, :SHARD // 2])
                nc.gpsimd.dma_start(x0A[:CIN, SHARD // 2:],
                                    x_d[0, :, SHARD // 2:])
            else:
                nc.sync.dma_start(x0A[:CIN, :], x_d[0, :, :])
            gcb0 = wts.tile([HID, 1], F32, tag="gcb0")
            nc.sync.dma_start(gcb0[:], gcb0_d[:])
            abr = wts.tile([HID, NB - 1], F32, tag="abr")
            gcbr = wts.tile([HID, NB - 1], F32, tag="gcbr")
            if not W_EARLY:
                nc.sync.dma_start(abr[:], abr_d[:])
                nc.sync.dma_start(gcbr[:], gcbr_d[:])
            epsb = wts.tile([HID, 1], F32, tag="epsb")
            nc.vector.memset(epsb[:], EPS * 0.5)
            # dummy Sqrt: pulls the ACT table load (1283ns) off style(0)'s
            # critical path; sqrt_and_others covers Identity/Prelu/Sqrt.
            warm = wts.tile([HID, 1], F32, tag="warm")
            nc.scalar.activation(warm[:], epsb[:],
                                 mybir.ActivationFunctionType.Sqrt)
            cTr = wts.tile([HID, NB - 1, HID], F32, tag="cTr")
            affTr = wts.tile([128, 4, NB - 1, HID], F32, tag="affTr")
            if W_EARLY:
                nc.sync.dma_start(cTr[:, 0, :], cTr_d[:, 0, :])
                nc.sync.dma_start(affTr[:, :, 0, :], affTr_d[:, :, 0, :])
                nc.sync.dma_start(abr[:], abr_d[:])
                nc.sync.dma_start(gcbr[:], gcbr_d[:])
                rest = range(1, NB - 1)
            else:
                rest = range(NB - 1)
            for _l in rest:
                nc.sync.dma_start(cTr[:, _l, :], cTr_d[:, _l, :])
                nc.sync.dma_start(affTr[:, :, _l, :], affTr_d[:, :, _l, :])

            # ---- style prep for all layers (tiny, runs up-front) ----
            def style(l):
                C = CIN if l == 0 else HID
                affT = (lambda j: affT0[:, j, :]) if l == 0 else (
                    lambda j: affTr[:, j, l - 1, :])
                ab = ab0[:, 0:1] if l == 0 else abr[:, l - 1:l]
                cT = cT0[:] if l == 0 else cTr[:, l - 1, :]

                ps_s = ps.tile([C, B], F32, tag="ps")
                for j in range(4):
                    nc.tensor.matmul(ps_s[:], affT(j), wpT[:, j, l, :],
                                     start=(j == 0), stop=(j == 3))
                sT = sty.tile([C, B], F32, tag="sT")
                nc.scalar.activation(sT[:], ps_s[:],
                                     mybir.ActivationFunctionType.Identity,
                                     bias=ab, scale=INV_SQRT_WDIM)
                ssq = sty.tile([C, B], F32, tag="ssq")
                nc.vector.tensor_mul(ssq[:], sT[:], sT[:])
                csq = sty.tile([C, HID], F32, tag="csq")
                nc.vector.tensor_mul(csq[:], cT, cT)
                ps_d = ps.tile([HID, B], F32, tag="ps")
                nc.tensor.matmul(ps_d[:], csq[:], ssq[:], start=True, stop=True)
                droot = sty.tile([HID, B], F32, tag="droot")
                nc.scalar.activation(droot[:], ps_d[:],
                                     mybir.ActivationFunctionType.Sqrt,
                                     bias=epsb[:, 0:1], scale=0.5)
                dscale = dscp.tile([HID, B], F32, tag="dscale")
                nc.vector.reciprocal(dscale[:], droot[:])
                wmod = wmodp.tile([C, B * HID], F32R, tag="wmod")
                for b in range(B):
                    nc.vector.tensor_scalar_mul(
                        wmod[:, b * HID:(b + 1) * HID], cT, sT[:, b:b + 1])
                return wmod, dscale

            def iteration(it):
                styles = [style(l) for l in range(NB)]
                # greedy time balance between the ACT and DVE epilogue queues
                # measured per-inst busy: ACT n*0.8333+185, DVE n*1.0417+125;
                # initial loads = fixed per-engine style work (ACT: sT/ssq/csq/
                # droot + table loads ~9.5us; DVE: wmod/recip ~4.5us)
                load = [9500.0, 4500.0]
                gcnt = [0]
                ACT_NS = lambda n: n * 0.8333 + 185
                DVE_NS = lambda n: n * 1.0417 + 125

                def load_block(blk):
                    b, col0, ncols = blk
                    if it == 0 and b == 0 and col0 == 0 and ncols == SHARD:
                        return x0A, x0B   # pre-issued before the weight DMAs
                    bufA = big.tile([128, ncols], F32R, tag="xbuf")
                    bufB = big.tile([128, ncols], F32R, tag="xbuf")
                    nc.sync.dma_start(bufA[:CIN, :],
                                      x_d[b, :, col0:col0 + ncols])
                    return bufA, bufB

                # column-blocked: load -> 8 layers -> store, prefetch next.
                # First/last batches split in half for shallower ramp/tail.
                if BLOCK_SPLIT:
                    blocks = ([(0, 0, SHARD // 2), (0, SHARD // 2, SHARD // 2)]
                              + [(b, 0, SHARD) for b in range(1, B - 1)]
                              + [(B - 1, 0, SHARD // 2),
                                 (B - 1, SHARD // 2, SHARD // 2)])
                elif SPLIT_LAST:
                    blocks = ([(b, 0, SHARD) for b in range(B - 1)]
                              + [(B - 1, 0, SHARD // 2),
                                 (B - 1, SHARD // 2, SHARD // 2)])
                else:
                    blocks = [(b, 0, SHARD) for b in range(B)]
                nxt = load_block(blocks[0])
                for bi, (b, col0, ncols) in enumerate(blocks):
                    bufA, bufB = nxt
                    if bi + 1 < len(blocks):
                        nxt = load_block(blocks[bi + 1])
                    for l in range(NB):
                        C = CIN if l == 0 else HID
                        gcb = gcb0[:, 0:1] if l == 0 else gcbr[:, l - 1:l]
                        x_in = bufA if l % 2 == 0 else bufB
                        x_out = bufB if l % 2 == 0 else bufA
                        last = l == NB - 1
                        wmod, dscale = styles[l]
                        for g in range(ncols // GROUP):
                            pt = ps.tile([128, GROUP], F32, tag="ps")
                            c0 = g * GROUP
                            for t in range(NT):
                                nc.tensor.matmul(
                                    pt[:, t * 512:(t + 1) * 512],
                                    wmod[:C, b * HID:(b + 1) * HID],
                                    x_in[:C, c0 + t * 512:c0 + (t + 1) * 512],
                                    start=True, stop=True)
                            # epilogue: out = prelu(psum*dscale + gcb, 0.2)
                            if last:
                                ost = xst.tile([128, GROUP], F32, tag="xout")
                                o_full = ost[:]
                            else:
                                o_full = x_out[:, c0:c0 + GROUP]
                            tA, tD = ACT_NS(GROUP), DVE_NS(GROUP)
                            if EPI_MODE == "act":
                                gi = 0
                            elif EPI_MODE == "dve":
                                gi = 1
                            elif EPI_BAL == "mod":
                                gi = 0 if (gcnt[0] * ACT_SHARE) % ACT_DEN < ACT_SHARE else 1
                                gcnt[0] += 1
                            else:
                                gi = 0 if load[0] + tA <= load[1] + tD else 1
                            if gi == 0:
                                load[0] += tA
                                nc.scalar.activation(
                                    o_full, pt[:],
                                    mybir.ActivationFunctionType.Prelu,
                                    bias=gcb, scale=dscale[:, b:b + 1],
                                    alpha=0.2)
                            else:
                                load[1] += tD
                                nc.vector._custom_dve(
                                    SCALE_BIAS_LRELU,
                                    out=o_full, in0=pt[:],
                                    s0=dscale[:, b:b + 1], s1=gcb,
                                    imm2=0.2)
                            if last:
                                dst = y_d[b, :, col0 + g * GROUP:
                                          col0 + (g + 1) * GROUP]
                                if STORE_Q == "alt":
                                    eng = nc.sync if g % 2 == 0 else nc.scalar
                                    eng.dma_start(dst, ost[:])
                                elif STORE_Q == "tail-alt" and bi == len(blocks) - 1:
                                    eng = nc.sync if g % 2 == 0 else nc.gpsimd
                                    eng.dma_start(dst, ost[:])
                                else:
                                    nc.gpsimd.dma_start(dst, ost[:])

            for it in range(K):
                iteration(it)

    nc.compile()
    return nc


def _prep_inputs(pre_point_features, points_encoding, wp,
                 aff_w_in, aff_b_in, conv_w_in, conv_b_in,
                 aff_w, aff_b, conv_w, conv_b):
    """Host-side layout prep (transposes/reshapes of small parameter tensors)."""
    x = np.ascontiguousarray(np.asarray(points_encoding, np.float32)
                             .reshape(B, CIN, HWTOT))
    wp = np.asarray(wp, np.float32)
    # wpT[p, j, l, b] = wp[b, l, j*128+p]
    wpT = np.ascontiguousarray(
        wp.transpose(2, 1, 0).reshape(4, 128, NB, B).transpose(1, 0, 2, 3))
    aff_w_in = np.asarray(aff_w_in, np.float32)
    affT0 = np.ascontiguousarray(
        aff_w_in.T.reshape(4, 128, CIN).transpose(1, 0, 2))
    aff_w = np.asarray(aff_w, np.float32)
    affTr = np.ascontiguousarray(
        aff_w.transpose(2, 0, 1).reshape(4, 128, NB - 1, HID).transpose(1, 0, 2, 3))
    ab0 = np.ascontiguousarray(np.asarray(aff_b_in, np.float32).reshape(CIN, 1))
    abr = np.ascontiguousarray(np.asarray(aff_b, np.float32).T)
    cT0 = np.ascontiguousarray(np.asarray(conv_w_in, np.float32).T)
    cTr = np.ascontiguousarray(np.asarray(conv_w, np.float32).transpose(2, 0, 1))
    gcb0 = np.ascontiguousarray(
        (SQRT2 * np.asarray(conv_b_in, np.float32)).reshape(HID, 1))
    gcbr = np.ascontiguousarray(SQRT2 * np.asarray(conv_b, np.float32).T)

    shared = dict(wpT=wpT, affT0=affT0, affTr=affTr, ab0=ab0, abr=abr,
                  cT0=cT0, cTr=cTr, gcb0=gcb0, gcbr=gcbr)
    in_maps = []
    for c in range(N_CORES):
        m = dict(shared)
        m["x"] = np.ascontiguousarray(x[:, :, c * SHARD:(c + 1) * SHARD])
        in_maps.append(m)
    return in_maps


def kernel(trace=False, **inputs):
    global _COMPILED
    if _COMPILED is None:
        _COMPILED = _build()
    nc = _COMPILED
    in_maps = _prep_inputs(**inputs)
    res = run_bass_kernel_spmd(nc, in_maps, core_ids=list(range(N_CORES)),
                               trace=trace)
    parts = [res.results[c]["y"] for c in range(N_CORES)]
    out = np.concatenate(parts, axis=2).reshape(B, HID, H, W)
    if trace:
        kernel.last_result = res
    return out



# revision 36
# speedup vs baseline: 188.6179x; 188.6179x over previous
"""Trainium2 Bass kernel for nn_NeRFMLPNetwork (StyleGAN-style modulated 1x1-conv MLP).

Network (per layer): s = affine(w_lat); y = conv1x1(x * s); y = y * rsqrt(demod) + b;
out = lrelu(y) * sqrt(2).  8 layers (60->128, then 7x 128->128), B=4, H*W=32768.

Strategy:
  - Data parallel over H*W: each of 8 cores handles 4096 spatial points (all batches).
  - Per (layer, batch) fold modulation s into the weight: Wmod[c,o] = convT[c,o]*s[b,c],
    kept in f32r (full-rate matmul, ~2^-13 precision).  Demod scale d and bias are
    applied in the epilogue: out = prelu(psum*dscale + sqrt2*cb, alpha=0.2), where
    dscale = sqrt(2/(v+eps)) folds in the sqrt(2) lrelu gain.
  - The epilogue (PSUM->SBUF pass over every output element) is the bottleneck:
    only ScalarE (1 elem/cyc @1.2GHz, Prelu) and VectorE (1 elem/cyc @0.96GHz,
    custom DVE op SCALE_BIAS_LRELU: out = max(z, 0.2z), z = in*s0+s1) can read
    PSUM, so groups alternate between them at a HW-calibrated 69:59 ratio
    (ACT ~1.04us, DVE ~1.25us per 1024-col group incl. access overheads).
  - Style path (tiny) on device: s via PE matmul over 4 K-chunks, demod sum via PE
    matmul of squared weights, sqrt on ScalarE, reciprocal on VectorE, squares
    and wmod scaling on VectorE.
  - DMA choreography matters: the SP sequencer issues DMAs serially (~650ns
    each), so style(0)'s weights go first, block-0 input rides the idle GPSIMD
    SWDGE queue, per-layer weights are packed into one DMA per layer, and the
    final block's stores alternate across two queues to shorten the tail.

Host-side prep is layout only: transposes/reshapes of the small parameter tensors
plus folding the constant sqrt(2) into the conv bias.
"""

import numpy as np

import concourse.bacc as bacc
import concourse.mybir as mybir
import concourse.tile as tile
from concourse.bass_utils import run_bass_kernel_spmd

# ---------------------------------------------------------------------------
# Custom DVE op: out = max(z, z*imm2) with z = in0*s0 + s1   (leaky relu)
# ---------------------------------------------------------------------------
import concourse.dve_ops as dve_ops_mod
from concourse.dve_spec import Spec, Src0, C0, C1, C2, maxx, lower as _dve_lower
from concourse.dve_spec import _has_src1
from concourse.dve_uop import DveOpSpec


def _sbl_ref(in0, in1, s0, s1, imm2):
    z = in0.astype(np.float32) * s0 + s1
    return np.maximum(z, z * imm2)


_z = Src0 * C0 + C1
_SBL_SPEC = Spec(body=maxx(_z, _z * C2), reference=_sbl_ref)
SCALE_BIAS_LRELU = dve_ops_mod.DveOp(
    "SCALE_BIAS_LRELU", _SBL_SPEC, subdim=False, uops_sha={}
)
if "SCALE_BIAS_LRELU" not in dve_ops_mod._SUB_OPCODE_FOR_NAME:
    dve_ops_mod.OPS.append(SCALE_BIAS_LRELU)
    dve_ops_mod.CUSTOM_DVE_SPECS["SCALE_BIAS_LRELU"] = _SBL_SPEC
    dve_ops_mod._SUB_OPCODE_FOR_NAME["SCALE_BIAS_LRELU"] = (
        max(dve_ops_mod._SUB_OPCODE_FOR_NAME.values()) + 1
    )
for _ver in ("v3", "v4"):
    _s = DveOpSpec(
        name="SCALE_BIAS_LRELU",
        opcode=dve_ops_mod.get_dve_sub_opcode("SCALE_BIAS_LRELU"),
        uops=_dve_lower(_SBL_SPEC, ver=_ver),
        rd1_en=_has_src1(_SBL_SPEC),
    )
    SCALE_BIAS_LRELU.uops_sha[_ver] = _s.sha(_ver)

# ---------------------------------------------------------------------------
# Problem constants (hardcoded per spec)
# ---------------------------------------------------------------------------
B, CIN, H, W, HID, WDIM, NB = 4, 60, 64, 512, 128, 512, 8
HWTOT = H * W                    # 32768
N_CORES = 8
SHARD = HWTOT // N_CORES         # 4096 spatial points per core
INV_SQRT_WDIM = float(1.0 / np.sqrt(WDIM))
SQRT2 = float(np.sqrt(2.0))
EPS = 1e-8

F32 = mybir.dt.float32
F32R = mybir.dt.float32r

GROUP = 1024                     # psum group columns (2 banks)
BLKCOLS = 4096                   # columns per processing block
SPLIT = 512                      # epilogue cols on ScalarE (bank-aligned); rest VectorE
NT = GROUP // 512                # matmuls per psum group
EPI_MODE = "split"               # 'split'(group-alternating) | 'splitcol' | 'act' | 'dve' | 'none'
ACT_SHARE = 69                   # of ACT_DEN groups go to ScalarE (rest VectorE)
ACT_DEN = 128
EPI_BAL = "mod"                  # 'greedy' | 'mod'
BLOCK_SPLIT = False              # split first/last batch into halves
SPLIT_LAST = False               # split only the last batch into halves
STORE_Q = "tail-alt"             # 'pool' | 'alt' | 'tail-alt'
X0_Q = "pool"                    # 'sync' | 'pool' (block-0 x via SWDGE, 2 chunks)
W_EARLY = False                  # cTr/affTr[0] + abr/gcbr before the rest
ST_ENG = "act"                   # 'act' | 'dve' (style affine engine)
PAIR = False                     # paired-batch waves (2 blocks per layer sweep)
DFOLD = False                    # fold dscale(l) into style(l+1) (needs conv_b[0:7]==0)
OFF_EVERY = 0                    # offload every Nth mid-layer group to DMA+gpsimd (0=off)

_COMPILED = None


def _build(K=1):
    """Build the program; K>1 unrolls the whole pipeline K times (for timing)."""
    nc = bacc.Bacc("TRN2", target_bir_lowering=False, debug=False,
                   num_devices=N_CORES)

    # x is declared f32r: raw f32 bits DMA directly; the PE rounds on read
    # (verified bit-identical to a DVE f32->f32r rounding copy).
    x_d = nc.dram_tensor("x", [B, CIN, SHARD], F32R, kind="ExternalInput").ap()
    wpT_d = nc.dram_tensor("wpT", [128, 4, NB, B], F32, kind="ExternalInput").ap()
    affT0_d = nc.dram_tensor("affT0", [128, 4, CIN], F32, kind="ExternalInput").ap()
    wpk_d = nc.dram_tensor("wpk", [128, NB - 1, 5, HID], F32, kind="ExternalInput").ap()
    ab0_d = nc.dram_tensor("ab0", [CIN, 1], F32, kind="ExternalInput").ap()
    abr_d = nc.dram_tensor("abr", [HID, NB - 1], F32, kind="ExternalInput").ap()
    cT0_d = nc.dram_tensor("cT0", [CIN, HID], F32, kind="ExternalInput").ap()
    gcb0_d = nc.dram_tensor("gcb0", [HID, 1], F32, kind="ExternalInput").ap()
    gcbr_d = nc.dram_tensor("gcbr", [HID, NB - 1], F32, kind="ExternalInput").ap()
    y_d = nc.dram_tensor("y", [B, HID, SHARD], F32, kind="ExternalOutput").ap()

    COLS = B * SHARD             # 16384 columns resident per core

    with tile.TileContext(nc) as tc:
        with (
            tc.tile_pool(name="big", bufs=8 if PAIR else 4) as big,
            tc.tile_pool(name="wts", bufs=1) as wts,
            tc.tile_pool(name="xst", bufs=4) as xst,
            tc.tile_pool(name="sty", bufs=3) as sty,
            tc.tile_pool(name="wmod", bufs=NB + 1) as wmodp,
            tc.tile_pool(name="dsc", bufs=NB + 1) as dscp,
            tc.tile_pool(name="ps", bufs=4, space="PSUM") as ps,
            tc.tile_pool(name="stg", bufs=3) as stgp,
        ):
            # ---- DMA order: style(0)'s weights first, then block-0 input,
            # then everything else.  The SP sequencer serializes DMA issues
            # at ~650ns each, so x0 must not queue behind the 16 per-layer
            # weight DMAs (that alone put the main-loop start at ~14us). ----
            wpT = wts.tile([128, 4, NB, B], F32, tag="wpT")
            nc.sync.dma_start(wpT[:], wpT_d[:])
            affT0 = wts.tile([128, 4, CIN], F32, tag="affT0")
            nc.sync.dma_start(affT0[:], affT0_d[:])
            ab0 = wts.tile([CIN, 1], F32, tag="ab0")
            nc.sync.dma_start(ab0[:], ab0_d[:])
            cT0 = wts.tile([CIN, HID], F32, tag="cT0")
            nc.sync.dma_start(cT0[:], cT0_d[:])
            x0A = big.tile([128, SHARD], F32R, tag="xbuf")
            x0B = big.tile([128, SHARD], F32R, tag="xbuf")
            if X0_Q == "pool":
                nc.gpsimd.dma_start(x0A[:CIN, :SHARD // 2],
                                    x_d[0, :, :SHARD // 2])
                nc.gpsimd.dma_start(x0A[:CIN, SHARD // 2:],
                                    x_d[0, :, SHARD // 2:])
            else:
                nc.sync.dma_start(x0A[:CIN, :], x_d[0, :, :])
            if PAIR:
                x1A = big.tile([128, SHARD], F32R, tag="xbuf")
                x1B = big.tile([128, SHARD], F32R, tag="xbuf")
                nc.gpsimd.dma_start(x1A[:CIN, :SHARD // 2],
                                    x_d[1, :, :SHARD // 2])
                nc.gpsimd.dma_start(x1A[:CIN, SHARD // 2:],
                                    x_d[1, :, SHARD // 2:])
            gcb0 = wts.tile([HID, 1], F32, tag="gcb0")
            nc.sync.dma_start(gcb0[:], gcb0_d[:])
            abr = wts.tile([HID, NB - 1], F32, tag="abr")
            gcbr = wts.tile([HID, NB - 1], F32, tag="gcbr")
            if not W_EARLY:
                nc.sync.dma_start(abr[:], abr_d[:])
                nc.sync.dma_start(gcbr[:], gcbr_d[:])
            epsb = wts.tile([HID, 1], F32, tag="epsb")
            nc.vector.memset(epsb[:], EPS * 0.5)
            # dummy Sqrt: pulls the ACT table load (1283ns) off style(0)'s
            # critical path; sqrt_and_others covers Identity/Prelu/Sqrt.
            warm = wts.tile([HID, 1], F32, tag="warm")
            nc.scalar.activation(warm[:], epsb[:],
                                 mybir.ActivationFunctionType.Sqrt)
            wpk = wts.tile([128, NB - 1, 5, HID], F32, tag="wpk")
            for _l in range(NB - 1):
                nc.sync.dma_start(wpk[:, _l, :, :], wpk_d[:, _l, :, :])

            # ---- style prep for all layers (tiny, runs up-front) ----
            def style(l, dsc_prev=None):
                C = CIN if l == 0 else HID
                affT = (lambda j: affT0[:, j, :]) if l == 0 else (
                    lambda j: wpk[:, l - 1, 1 + j, :])
                ab = ab0[:, 0:1] if l == 0 else abr[:, l - 1:l]
                cT = cT0[:] if l == 0 else wpk[:, l - 1, 0, :]

                ps_s = ps.tile([C, B], F32, tag="ps")
                for j in range(4):
                    nc.tensor.matmul(ps_s[:], affT(j), wpT[:, j, l, :],
                                     start=(j == 0), stop=(j == 3))
                sT = sty.tile([C, B], F32, tag="sT")
                if ST_ENG == "dve":
                    nc.vector.tensor_scalar(sT[:], ps_s[:], INV_SQRT_WDIM, ab,
                                            mybir.AluOpType.mult,
                                            mybir.AluOpType.add)
                else:
                    nc.scalar.activation(sT[:], ps_s[:],
                                         mybir.ActivationFunctionType.Identity,
                                         bias=ab, scale=INV_SQRT_WDIM)
                ssq = sty.tile([C, B], F32, tag="ssq")
                nc.vector.tensor_mul(ssq[:], sT[:], sT[:])
                csq = sty.tile([C, HID], F32, tag="csq")
                nc.vector.tensor_mul(csq[:], cT, cT)
                ps_d = ps.tile([HID, B], F32, tag="ps")
                nc.tensor.matmul(ps_d[:], csq[:], ssq[:], start=True, stop=True)
                droot = sty.tile([HID, B], F32, tag="droot")
                nc.scalar.activation(droot[:], ps_d[:],
                                     mybir.ActivationFunctionType.Sqrt,
                                     bias=epsb[:, 0:1], scale=0.5)
                dscale = dscp.tile([HID, B], F32, tag="dscale")
                nc.vector.reciprocal(dscale[:], droot[:])
                wmod = wmodp.tile([C, B * HID], F32R, tag="wmod")
                for b in range(B):
                    nc.vector.tensor_scalar_mul(
                        wmod[:, b * HID:(b + 1) * HID], cT, sT[:, b:b + 1])
                return wmod, dscale

            def iteration(it):
                styles = []
                for l in range(NB):
                    prev = styles[l - 1][1] if l > 0 else None
                    styles.append(style(l, prev))
                # greedy time balance between the ACT and DVE epilogue queues
                # measured per-inst busy: ACT n*0.8333+185, DVE n*1.0417+125;
                # initial loads = fixed per-engine style work (ACT: sT/ssq/csq/
                # droot + table loads ~9.5us; DVE: wmod/recip ~4.5us)
                load = [9500.0, 4500.0]
                gcnt = [0]
                ocnt = [0]
                ACT_NS = lambda n: n * 0.8333 + 185
                DVE_NS = lambda n: n * 1.0417 + 125

                def load_block(blk):
                    b, col0, ncols = blk
                    if it == 0 and b == 0 and col0 == 0 and ncols == SHARD:
                        return x0A, x0B   # pre-issued before the weight DMAs
                    bufA = big.tile([128, ncols], F32R, tag="xbuf")
                    bufB = big.tile([128, ncols], F32R, tag="xbuf")
                    nc.sync.dma_start(bufA[:CIN, :],
                                      x_d[b, :, col0:col0 + ncols])
                    return bufA, bufB

                def emit_group(b, col0, g, l, bufA, bufB, tail):
                    C = CIN if l == 0 else HID
                    gcb = gcb0[:, 0:1] if l == 0 else gcbr[:, l - 1:l]
                    x_in = bufA if l % 2 == 0 else bufB
                    x_out = bufB if l % 2 == 0 else bufA
                    last = l == NB - 1
                    wmod, dscale = styles[l]
                    pt = ps.tile([128, GROUP], F32, tag="ps")
                    c0 = g * GROUP
                    for t in range(NT):
                        nc.tensor.matmul(
                            pt[:, t * 512:(t + 1) * 512],
                            wmod[:C, b * HID:(b + 1) * HID],
                            x_in[:C, c0 + t * 512:c0 + (t + 1) * 512],
                            start=True, stop=True)
                    # epilogue: out = prelu(psum*dscale + gcb, 0.2)
                    if last:
                        ost = xst.tile([128, GROUP], F32, tag="xout")
                        o_full = ost[:]
                    else:
                        o_full = x_out[:, c0:c0 + GROUP]
                    folded = DFOLD and not last
                    tA, tD = ACT_NS(GROUP), DVE_NS(GROUP)
                    if EPI_MODE == "act":
                        gi = 0
                    elif EPI_MODE == "dve":
                        gi = 1
                    elif EPI_BAL == "mod":
                        gi = 0 if (gcnt[0] * ACT_SHARE) % ACT_DEN < ACT_SHARE else 1
                        gcnt[0] += 1
                    else:
                        gi = 0 if load[0] + tA <= load[1] + tD else 1
                    if folded and OFF_EVERY:
                        ocnt[0] += 1
                        if ocnt[0] % OFF_EVERY == 0:
                            gi = 2
                    if gi == 2:
                        # 3rd drain path: DMA psum->SBUF staging, then leaky
                        # relu on GPSIMD (max(0.2z, z); scale/bias not needed
                        # on folded layers)
                        stg = stgp.tile([128, GROUP], F32, tag="stg")
                        nc.sync.dma_start(stg[:], pt[:])
                        nc.gpsimd.scalar_tensor_tensor(
                            o_full, stg[:], 0.2, stg[:],
                            mybir.AluOpType.mult, mybir.AluOpType.max)
                    elif gi == 0:
                        load[0] += tA
                        nc.scalar.activation(
                            o_full, pt[:],
                            mybir.ActivationFunctionType.Prelu,
                            bias=0.0 if folded else gcb,
                            scale=1.0 if folded else dscale[:, b:b + 1],
                            alpha=0.2)
                    else:
                        load[1] += tD
                        if folded:
                            nc.vector._custom_dve(
                                SCALE_BIAS_LRELU,
                                out=o_full, in0=pt[:],
                                s0=1.0, s1=0.0, imm2=0.2)
                        else:
                            nc.vector._custom_dve(
                                SCALE_BIAS_LRELU,
                                out=o_full, in0=pt[:],
                                s0=dscale[:, b:b + 1], s1=gcb,
                                imm2=0.2)
                    if last:
                        dst = y_d[b, :, col0 + g * GROUP:
                                  col0 + (g + 1) * GROUP]
                        if STORE_Q == "alt":
                            eng = nc.sync if g % 2 == 0 else nc.scalar
                            eng.dma_start(dst, ost[:])
                        elif STORE_Q == "tail-alt" and tail:
                            eng = nc.sync if g % 2 == 0 else nc.gpsimd
                            eng.dma_start(dst, ost[:])
                        else:
                            nc.gpsimd.dma_start(dst, ost[:])

                if PAIR:
                    # waves of 2 batches advancing layer-by-layer together:
                    # styles only need to keep a 2-layer-per-wave cadence and
                    # layer-boundary pipeline refills amortize over 8 groups.
                    waves = [[0, 1], [2, 3]]
                    bufs = {}
                    if it == 0:
                        bufs[0] = (x0A, x0B)     # pre-issued on the pool queue
                        bufs[1] = (x1A, x1B)
                    else:
                        bufs[0] = load_block((0, 0, SHARD))
                        bufs[1] = load_block((1, 0, SHARD))
                    for wi, wave in enumerate(waves):
                        if wi + 1 < len(waves):
                            for b2 in waves[wi + 1]:
                                bufs[b2] = load_block((b2, 0, SHARD))
                        for l in range(NB):
                            for b in wave:
                                bufA, bufB = bufs[b]
                                for g in range(SHARD // GROUP):
                                    emit_group(b, 0, g, l, bufA, bufB,
                                               wi == len(waves) - 1)
                else:
                    if SPLIT_LAST:
                        blocks = ([(b, 0, SHARD) for b in range(B - 1)]
                                  + [(B - 1, 0, SHARD // 2),
                                     (B - 1, SHARD // 2, SHARD // 2)])
                    else:
                        blocks = [(b, 0, SHARD) for b in range(B)]
                    nxt = load_block(blocks[0])
                    for bi, (b, col0, ncols) in enumerate(blocks):
                        bufA, bufB = nxt
                        if bi + 1 < len(blocks):
                            nxt = load_block(blocks[bi + 1])
                        for l in range(NB):
                            for g in range(ncols // GROUP):
                                emit_group(b, col0, g, l, bufA, bufB,
                                           bi == len(blocks) - 1)

            for it in range(K):
                iteration(it)

    nc.compile()
    return nc


def _prep_inputs(pre_point_features, points_encoding, wp,
                 aff_w_in, aff_b_in, conv_w_in, conv_b_in,
                 aff_w, aff_b, conv_w, conv_b):
    """Host-side layout prep (transposes/reshapes of small parameter tensors)."""
    x = np.ascontiguousarray(np.asarray(points_encoding, np.float32)
                             .reshape(B, CIN, HWTOT))
    wp = np.asarray(wp, np.float32)
    # wpT[p, j, l, b] = wp[b, l, j*128+p]
    wpT = np.ascontiguousarray(
        wp.transpose(2, 1, 0).reshape(4, 128, NB, B).transpose(1, 0, 2, 3))
    aff_w_in = np.asarray(aff_w_in, np.float32)
    affT0 = np.ascontiguousarray(
        aff_w_in.T.reshape(4, 128, CIN).transpose(1, 0, 2))
    aff_w = np.asarray(aff_w, np.float32)
    # affTr[p, j, l, o] = aff_w[l, o, j*128+p]
    affTr = aff_w.transpose(2, 0, 1).reshape(4, 128, NB - 1, HID).transpose(1, 0, 2, 3)
    ab0 = np.ascontiguousarray(np.asarray(aff_b_in, np.float32).reshape(CIN, 1))
    abr = np.ascontiguousarray(np.asarray(aff_b, np.float32).T)
    cT0 = np.ascontiguousarray(np.asarray(conv_w_in, np.float32).T)
    cTr = np.asarray(conv_w, np.float32).transpose(2, 0, 1)
    # wpk[p, l, 0, :] = cTr[p, l, :]; wpk[p, l, 1+j, :] = affTr[p, j, l, :]
    wpk = np.empty((128, NB - 1, 5, HID), np.float32)
    wpk[:, :, 0, :] = cTr
    wpk[:, :, 1:, :] = affTr.transpose(0, 2, 1, 3)
    wpk = np.ascontiguousarray(wpk)
    gcb0 = np.ascontiguousarray(
        (SQRT2 * np.asarray(conv_b_in, np.float32)).reshape(HID, 1))
    gcbr = np.ascontiguousarray(SQRT2 * np.asarray(conv_b, np.float32).T)

    shared = dict(wpT=wpT, affT0=affT0, wpk=wpk, ab0=ab0, abr=abr,
                  cT0=cT0, gcb0=gcb0, gcbr=gcbr)
    in_maps = []
    for c in range(N_CORES):
        m = dict(shared)
        m["x"] = np.ascontiguousarray(x[:, :, c * SHARD:(c + 1) * SHARD])
        in_maps.append(m)
    return in_maps


def kernel(trace=False, **inputs):
    global _COMPILED
    if _COMPILED is None:
        _COMPILED = _build()
    nc = _COMPILED
    in_maps = _prep_inputs(**inputs)
    res = run_bass_kernel_spmd(nc, in_maps, core_ids=list(range(N_CORES)),
                               trace=trace)
    parts = [res.results[c]["y"] for c in range(N_CORES)]
    out = np.concatenate(parts, axis=2).reshape(B, HID, H, W)
    if trace:
        kernel.last_result = res
    return out



# revision 40
# speedup vs baseline: 189.1114x; 1.0026x over previous
"""Trainium2 Bass kernel for nn_NeRFMLPNetwork (StyleGAN-style modulated 1x1-conv MLP).

Network (per layer): s = affine(w_lat); y = conv1x1(x * s); y = y * rsqrt(demod) + b;
out = lrelu(y) * sqrt(2).  8 layers (60->128, then 7x 128->128), B=4, H*W=32768.

Strategy:
  - Data parallel over H*W: each of 8 cores handles 4096 spatial points (all batches).
  - Per (layer, batch) fold modulation s into the weight: Wmod[c,o] = convT[c,o]*s[b,c],
    kept in f32r (full-rate matmul, ~2^-13 precision).  Demod scale d and bias are
    applied in the epilogue: out = prelu(psum*dscale + sqrt2*cb, alpha=0.2), where
    dscale = sqrt(2/(v+eps)) folds in the sqrt(2) lrelu gain.
  - The epilogue (PSUM->SBUF pass over every output element) is the bottleneck:
    only ScalarE (1 elem/cyc @1.2GHz, Prelu) and VectorE (1 elem/cyc @0.96GHz,
    custom DVE op SCALE_BIAS_LRELU: out = max(z, 0.2z), z = in*s0+s1) can read
    PSUM, so groups alternate between them at a HW-calibrated 69:59 ratio
    (ACT ~1.04us, DVE ~1.25us per 1024-col group incl. access overheads).
  - Style path (tiny) on device: s via PE matmul over 4 K-chunks, demod sum via PE
    matmul of squared weights, sqrt on ScalarE, reciprocal on VectorE, squares
    and wmod scaling on VectorE.
  - DMA choreography matters: the SP sequencer issues DMAs serially (~650ns
    each), so style(0)'s weights go first, block-0 input rides the idle GPSIMD
    SWDGE queue, per-layer weights are packed into one DMA per layer, and the
    final block's stores alternate across two queues to shorten the tail.

Host-side prep is layout only: transposes/reshapes of the small parameter tensors
plus folding the constant sqrt(2) into the conv bias.
"""

import numpy as np

import concourse.bacc as bacc
import concourse.mybir as mybir
import concourse.tile as tile
from concourse.bass_utils import run_bass_kernel_spmd

# ---------------------------------------------------------------------------
# Custom DVE op: out = max(z, z*imm2) with z = in0*s0 + s1   (leaky relu)
# ---------------------------------------------------------------------------
import concourse.dve_ops as dve_ops_mod
from concourse.dve_spec import Spec, Src0, C0, C1, C2, maxx, lower as _dve_lower
from concourse.dve_spec import _has_src1
from concourse.dve_uop import DveOpSpec


def _sbl_ref(in0, in1, s0, s1, imm2):
    z = in0.astype(np.float32) * s0 + s1
    return np.maximum(z, z * imm2)


_z = Src0 * C0 + C1
_SBL_SPEC = Spec(body=maxx(_z, _z * C2), reference=_sbl_ref)
SCALE_BIAS_LRELU = dve_ops_mod.DveOp(
    "SCALE_BIAS_LRELU", _SBL_SPEC, subdim=False, uops_sha={}
)
if "SCALE_BIAS_LRELU" not in dve_ops_mod._SUB_OPCODE_FOR_NAME:
    dve_ops_mod.OPS.append(SCALE_BIAS_LRELU)
    dve_ops_mod.CUSTOM_DVE_SPECS["SCALE_BIAS_LRELU"] = _SBL_SPEC
    dve_ops_mod._SUB_OPCODE_FOR_NAME["SCALE_BIAS_LRELU"] = (
        max(dve_ops_mod._SUB_OPCODE_FOR_NAME.values()) + 1
    )
for _ver in ("v3", "v4"):
    _s = DveOpSpec(
        name="SCALE_BIAS_LRELU",
        opcode=dve_ops_mod.get_dve_sub_opcode("SCALE_BIAS_LRELU"),
        uops=_dve_lower(_SBL_SPEC, ver=_ver),
        rd1_en=_has_src1(_SBL_SPEC),
    )
    SCALE_BIAS_LRELU.uops_sha[_ver] = _s.sha(_ver)

# ---------------------------------------------------------------------------
# Problem constants (hardcoded per spec)
# ---------------------------------------------------------------------------
B, CIN, H, W, HID, WDIM, NB = 4, 60, 64, 512, 128, 512, 8
HWTOT = H * W                    # 32768
N_CORES = 8
SHARD = HWTOT // N_CORES         # 4096 spatial points per core
INV_SQRT_WDIM = float(1.0 / np.sqrt(WDIM))
SQRT2 = float(np.sqrt(2.0))
EPS = 1e-8

F32 = mybir.dt.float32
F32R = mybir.dt.float32r

GROUP = 1024                     # psum group columns (2 banks)
BLKCOLS = 4096                   # columns per processing block
SPLIT = 512                      # epilogue cols on ScalarE (bank-aligned); rest VectorE
NT = GROUP // 512                # matmuls per psum group
EPI_MODE = "split"               # 'split'(group-alternating) | 'splitcol' | 'act' | 'dve' | 'none'
ACT_SHARE = 69                   # of ACT_DEN groups go to ScalarE (rest VectorE)
ACT_DEN = 128
EPI_BAL = "mod"                  # 'greedy' | 'mod'
BLOCK_SPLIT = False              # split first/last batch into halves
SPLIT_LAST = False               # split only the last batch into halves
STORE_Q = "tail-sync"            # 'pool' | 'alt' | 'tail-alt'
X0_Q = "pool"                    # 'sync' | 'pool' (block-0 x via SWDGE, 2 chunks)
W_EARLY = False                  # cTr/affTr[0] + abr/gcbr before the rest
WPK_Q = "sync"                   # 'sync' | 'alt' (alternate per-layer weight DMAs)
ST_ENG = "act"                   # 'act' | 'dve' (style affine engine)
PAIR = False                     # paired-batch waves (2 blocks per layer sweep)
DFOLD = False                    # fold dscale(l) into style(l+1) (needs conv_b[0:7]==0)
OFF_EVERY = 0                    # offload every Nth mid-layer group to DMA+gpsimd (0=off)

_COMPILED = None


def _build(K=1):
    """Build the program; K>1 unrolls the whole pipeline K times (for timing)."""
    # DFOLD's style-side fold is not implemented; the epilogue-side branches
    # would silently drop the demod scale. OFF_EVERY needs DFOLD (and DMA
    # cannot read PSUM on TRN2 anyway).
    assert not DFOLD and OFF_EVERY == 0
    nc = bacc.Bacc("TRN2", target_bir_lowering=False, debug=False,
                   num_devices=N_CORES)

    # x is declared f32r: raw f32 bits DMA directly; the PE rounds on read
    # (verified bit-identical to a DVE f32->f32r rounding copy).
    x_d = nc.dram_tensor("x", [B, CIN, SHARD], F32R, kind="ExternalInput").ap()
    wpT_d = nc.dram_tensor("wpT", [128, 4, NB, B], F32, kind="ExternalInput").ap()
    affT0_d = nc.dram_tensor("affT0", [128, 4, CIN], F32, kind="ExternalInput").ap()
    wpk_d = nc.dram_tensor("wpk", [128, NB - 1, 5, HID], F32, kind="ExternalInput").ap()
    ab0_d = nc.dram_tensor("ab0", [CIN, 1], F32, kind="ExternalInput").ap()
    abr_d = nc.dram_tensor("abr", [HID, NB - 1], F32, kind="ExternalInput").ap()
    cT0_d = nc.dram_tensor("cT0", [CIN, HID], F32, kind="ExternalInput").ap()
    gcb0_d = nc.dram_tensor("gcb0", [HID, 1], F32, kind="ExternalInput").ap()
    gcbr_d = nc.dram_tensor("gcbr", [HID, NB - 1], F32, kind="ExternalInput").ap()
    y_d = nc.dram_tensor("y", [B, HID, SHARD], F32, kind="ExternalOutput").ap()

    COLS = B * SHARD             # 16384 columns resident per core

    with tile.TileContext(nc) as tc:
        with (
            tc.tile_pool(name="big", bufs=8 if PAIR else 4) as big,
            tc.tile_pool(name="wts", bufs=1) as wts,
            tc.tile_pool(name="xst", bufs=4) as xst,
            tc.tile_pool(name="sty", bufs=3) as sty,
            tc.tile_pool(name="wmod", bufs=NB + 1) as wmodp,
            tc.tile_pool(name="dsc", bufs=NB + 1) as dscp,
            tc.tile_pool(name="ps", bufs=4, space="PSUM") as ps,
            tc.tile_pool(name="stg", bufs=3) as stgp,
        ):
            # ---- DMA order: style(0)'s weights first, then block-0 input,
            # then everything else.  The SP sequencer serializes DMA issues
            # at ~650ns each, so x0 must not queue behind the 16 per-layer
            # weight DMAs (that alone put the main-loop start at ~14us). ----
            wpT = wts.tile([128, 4, NB, B], F32, tag="wpT")
            nc.sync.dma_start(wpT[:], wpT_d[:])
            affT0 = wts.tile([128, 4, CIN], F32, tag="affT0")
            nc.sync.dma_start(affT0[:], affT0_d[:])
            ab0 = wts.tile([CIN, 1], F32, tag="ab0")
            nc.sync.dma_start(ab0[:], ab0_d[:])
            cT0 = wts.tile([CIN, HID], F32, tag="cT0")
            nc.sync.dma_start(cT0[:], cT0_d[:])
            x0A = big.tile([128, SHARD], F32R, tag="xbuf")
            x0B = big.tile([128, SHARD], F32R, tag="xbuf")
            if X0_Q == "pool":
                nc.gpsimd.dma_start(x0A[:CIN, :SHARD // 2],
                                    x_d[0, :, :SHARD // 2])
                nc.gpsimd.dma_start(x0A[:CIN, SHARD // 2:],
                                    x_d[0, :, SHARD // 2:])
            else:
                nc.sync.dma_start(x0A[:CIN, :], x_d[0, :, :])
            if PAIR:
                x1A = big.tile([128, SHARD], F32R, tag="xbuf")
                x1B = big.tile([128, SHARD], F32R, tag="xbuf")
                nc.gpsimd.dma_start(x1A[:CIN, :SHARD // 2],
                                    x_d[1, :, :SHARD // 2])
                nc.gpsimd.dma_start(x1A[:CIN, SHARD // 2:],
                                    x_d[1, :, SHARD // 2:])
            gcb0 = wts.tile([HID, 1], F32, tag="gcb0")
            nc.sync.dma_start(gcb0[:], gcb0_d[:])
            abr = wts.tile([HID, NB - 1], F32, tag="abr")
            gcbr = wts.tile([HID, NB - 1], F32, tag="gcbr")
            if not W_EARLY:
                nc.sync.dma_start(abr[:], abr_d[:])
                nc.sync.dma_start(gcbr[:], gcbr_d[:])
            epsb = wts.tile([HID, 1], F32, tag="epsb")
            nc.vector.memset(epsb[:], EPS * 0.5)
            # dummy Sqrt: pulls the ACT table load (1283ns) off style(0)'s
            # critical path; sqrt_and_others covers Identity/Prelu/Sqrt.
            warm = wts.tile([HID, 1], F32, tag="warm")
            nc.scalar.activation(warm[:], epsb[:],
                                 mybir.ActivationFunctionType.Sqrt)
            wpk = wts.tile([128, NB - 1, 5, HID], F32, tag="wpk")
            for _l in range(NB - 1):
                eng = nc.sync if (_l % 2 == 0 or WPK_Q != "alt") else nc.scalar
                eng.dma_start(wpk[:, _l, :, :], wpk_d[:, _l, :, :])

            # ---- style prep for all layers (tiny, runs up-front) ----
            def style(l, dsc_prev=None):
                C = CIN if l == 0 else HID
                affT = (lambda j: affT0[:, j, :]) if l == 0 else (
                    lambda j: wpk[:, l - 1, 1 + j, :])
                ab = ab0[:, 0:1] if l == 0 else abr[:, l - 1:l]
                cT = cT0[:] if l == 0 else wpk[:, l - 1, 0, :]

                ps_s = ps.tile([C, B], F32, tag="ps")
                for j in range(4):
                    nc.tensor.matmul(ps_s[:], affT(j), wpT[:, j, l, :],
                                     start=(j == 0), stop=(j == 3))
                sT = sty.tile([C, B], F32, tag="sT")
                if ST_ENG == "dve":
                    nc.vector.tensor_scalar(sT[:], ps_s[:], INV_SQRT_WDIM, ab,
                                            mybir.AluOpType.mult,
                                            mybir.AluOpType.add)
                else:
                    nc.scalar.activation(sT[:], ps_s[:],
                                         mybir.ActivationFunctionType.Identity,
                                         bias=ab, scale=INV_SQRT_WDIM)
                ssq = sty.tile([C, B], F32, tag="ssq")
                nc.vector.tensor_mul(ssq[:], sT[:], sT[:])
                csq = sty.tile([C, HID], F32, tag="csq")
                nc.vector.tensor_mul(csq[:], cT, cT)
                ps_d = ps.tile([HID, B], F32, tag="ps")
                nc.tensor.matmul(ps_d[:], csq[:], ssq[:], start=True, stop=True)
                droot = sty.tile([HID, B], F32, tag="droot")
                nc.scalar.activation(droot[:], ps_d[:],
                                     mybir.ActivationFunctionType.Sqrt,
                                     bias=epsb[:, 0:1], scale=0.5)
                dscale = dscp.tile([HID, B], F32, tag="dscale")
                nc.vector.reciprocal(dscale[:], droot[:])
                wmod = wmodp.tile([C, B * HID], F32R, tag="wmod")
                for b in range(B):
                    nc.vector.tensor_scalar_mul(
                        wmod[:, b * HID:(b + 1) * HID], cT, sT[:, b:b + 1])
                return wmod, dscale

            def iteration(it):
                styles = []
                for l in range(NB):
                    prev = styles[l - 1][1] if l > 0 else None
                    styles.append(style(l, prev))
                # greedy time balance between the ACT and DVE epilogue queues
                # measured per-inst busy: ACT n*0.8333+185, DVE n*1.0417+125;
                # initial loads = fixed per-engine style work (ACT: sT/ssq/csq/
                # droot + table loads ~9.5us; DVE: wmod/recip ~4.5us)
                load = [9500.0, 4500.0]
                gcnt = [0]
                ocnt = [0]
                ACT_NS = lambda n: n * 0.8333 + 185
                DVE_NS = lambda n: n * 1.0417 + 125

                def load_block(blk):
                    b, col0, ncols = blk
                    if it == 0 and b == 0 and col0 == 0 and ncols == SHARD:
                        return x0A, x0B   # pre-issued before the weight DMAs
                    bufA = big.tile([128, ncols], F32R, tag="xbuf")
                    bufB = big.tile([128, ncols], F32R, tag="xbuf")
                    nc.sync.dma_start(bufA[:CIN, :],
                                      x_d[b, :, col0:col0 + ncols])
                    return bufA, bufB

                def emit_group(b, col0, g, l, bufA, bufB, tail):
                    C = CIN if l == 0 else HID
                    gcb = gcb0[:, 0:1] if l == 0 else gcbr[:, l - 1:l]
                    x_in = bufA if l % 2 == 0 else bufB
                    x_out = bufB if l % 2 == 0 else bufA
                    last = l == NB - 1
                    wmod, dscale = styles[l]
                    pt = ps.tile([128, GROUP], F32, tag="ps")
                    c0 = g * GROUP
                    for t in range(NT):
                        nc.tensor.matmul(
                            pt[:, t * 512:(t + 1) * 512],
                            wmod[:C, b * HID:(b + 1) * HID],
                            x_in[:C, c0 + t * 512:c0 + (t + 1) * 512],
                            start=True, stop=True)
                    # epilogue: out = prelu(psum*dscale + gcb, 0.2)
                    if last:
                        ost = xst.tile([128, GROUP], F32, tag="xout")
                        o_full = ost[:]
                    else:
                        o_full = x_out[:, c0:c0 + GROUP]
                    folded = DFOLD and not last
                    tA, tD = ACT_NS(GROUP), DVE_NS(GROUP)
                    if EPI_MODE == "act":
                        gi = 0
                    elif EPI_MODE == "dve":
                        gi = 1
                    elif EPI_BAL == "mod":
                        gi = 0 if (gcnt[0] * ACT_SHARE) % ACT_DEN < ACT_SHARE else 1
                        gcnt[0] += 1
                    else:
                        gi = 0 if load[0] + tA <= load[1] + tD else 1
                    if folded and OFF_EVERY:
                        ocnt[0] += 1
                        if ocnt[0] % OFF_EVERY == 0:
                            gi = 2
                    if gi == 2:
                        # 3rd drain path: DMA psum->SBUF staging, then leaky
                        # relu on GPSIMD (max(0.2z, z); scale/bias not needed
                        # on folded layers)
                        stg = stgp.tile([128, GROUP], F32, tag="stg")
                        nc.sync.dma_start(stg[:], pt[:])
                        nc.gpsimd.scalar_tensor_tensor(
                            o_full, stg[:], 0.2, stg[:],
                            mybir.AluOpType.mult, mybir.AluOpType.max)
                    elif gi == 0:
                        load[0] += tA
                        nc.scalar.activation(
                            o_full, pt[:],
                            mybir.ActivationFunctionType.Prelu,
                            bias=0.0 if folded else gcb,
                            scale=1.0 if folded else dscale[:, b:b + 1],
                            alpha=0.2)
                    else:
                        load[1] += tD
                        if folded:
                            nc.vector._custom_dve(
                                SCALE_BIAS_LRELU,
                                out=o_full, in0=pt[:],
                                s0=1.0, s1=0.0, imm2=0.2)
                        else:
                            nc.vector._custom_dve(
                                SCALE_BIAS_LRELU,
                                out=o_full, in0=pt[:],
                                s0=dscale[:, b:b + 1], s1=gcb,
                                imm2=0.2)
                    if last:
                        dst = y_d[b, :, col0 + g * GROUP:
                                  col0 + (g + 1) * GROUP]
                        if STORE_Q == "alt":
                            eng = nc.sync if g % 2 == 0 else nc.scalar
                            eng.dma_start(dst, ost[:])
                        elif STORE_Q == "tail-alt" and tail:
                            eng = nc.sync if g % 2 == 0 else nc.gpsimd
                            eng.dma_start(dst, ost[:])
                        elif STORE_Q == "tail-sync" and tail:
                            nc.sync.dma_start(dst, ost[:])
                        elif STORE_Q == "tail-sync2" and tail:
                            # split each last-block store in half across the
                            # two queues so transfers interleave
                            h = GROUP // 2
                            nc.sync.dma_start(dst[:, :h], ost[:, :h])
                            nc.gpsimd.dma_start(dst[:, h:], ost[:, h:])
                        else:
                            nc.gpsimd.dma_start(dst, ost[:])

                if PAIR:
                    # waves of 2 batches advancing layer-by-layer together:
                    # styles only need to keep a 2-layer-per-wave cadence and
                    # layer-boundary pipeline refills amortize over 8 groups.
                    waves = [[0, 1], [2, 3]]
                    bufs = {}
                    if it == 0:
                        bufs[0] = (x0A, x0B)     # pre-issued on the pool queue
                        bufs[1] = (x1A, x1B)
                    else:
                        bufs[0] = load_block((0, 0, SHARD))
                        bufs[1] = load_block((1, 0, SHARD))
                    for wi, wave in enumerate(waves):
                        if wi + 1 < len(waves):
                            for b2 in waves[wi + 1]:
                                bufs[b2] = load_block((b2, 0, SHARD))
                        for l in range(NB):
                            for b in wave:
                                bufA, bufB = bufs[b]
                                for g in range(SHARD // GROUP):
                                    emit_group(b, 0, g, l, bufA, bufB,
                                               wi == len(waves) - 1)
                else:
                    if SPLIT_LAST:
                        blocks = ([(b, 0, SHARD) for b in range(B - 1)]
                                  + [(B - 1, 0, SHARD // 2),
                                     (B - 1, SHARD // 2, SHARD // 2)])
                    else:
                        blocks = [(b, 0, SHARD) for b in range(B)]
                    nxt = load_block(blocks[0])
                    for bi, (b, col0, ncols) in enumerate(blocks):
                        bufA, bufB = nxt
                        if bi + 1 < len(blocks):
                            nxt = load_block(blocks[bi + 1])
                        for l in range(NB):
                            for g in range(ncols // GROUP):
                                emit_group(b, col0, g, l, bufA, bufB,
                                           bi == len(blocks) - 1)

            for it in range(K):
                iteration(it)

    nc.compile()
    return nc


def _prep_inputs(pre_point_features, points_encoding, wp,
                 aff_w_in, aff_b_in, conv_w_in, conv_b_in,
                 aff_w, aff_b, conv_w, conv_b):
    """Host-side layout prep (transposes/reshapes of small parameter tensors)."""
    x = np.ascontiguousarray(np.asarray(points_encoding, np.float32)
                             .reshape(B, CIN, HWTOT))
    wp = np.asarray(wp, np.float32)
    # wpT[p, j, l, b] = wp[b, l, j*128+p]
    wpT = np.ascontiguousarray(
        wp.transpose(2, 1, 0).reshape(4, 128, NB, B).transpose(1, 0, 2, 3))
    aff_w_in = np.asarray(aff_w_in, np.float32)
    affT0 = np.ascontiguousarray(
        aff_w_in.T.reshape(4, 128, CIN).transpose(1, 0, 2))
    aff_w = np.asarray(aff_w, np.float32)
    # affTr[p, j, l, o] = aff_w[l, o, j*128+p]
    affTr = aff_w.transpose(2, 0, 1).reshape(4, 128, NB - 1, HID).transpose(1, 0, 2, 3)
    ab0 = np.ascontiguousarray(np.asarray(aff_b_in, np.float32).reshape(CIN, 1))
    abr = np.ascontiguousarray(np.asarray(aff_b, np.float32).T)
    cT0 = np.ascontiguousarray(np.asarray(conv_w_in, np.float32).T)
    cTr = np.asarray(conv_w, np.float32).transpose(2, 0, 1)
    # wpk[p, l, 0, :] = cTr[p, l, :]; wpk[p, l, 1+j, :] = affTr[p, j, l, :]
    wpk = np.empty((128, NB - 1, 5, HID), np.float32)
    wpk[:, :, 0, :] = cTr
    wpk[:, :, 1:, :] = affTr.transpose(0, 2, 1, 3)
    wpk = np.ascontiguousarray(wpk)
    gcb0 = np.ascontiguousarray(
        (SQRT2 * np.asarray(conv_b_in, np.float32)).reshape(HID, 1))
    gcbr = np.ascontiguousarray(SQRT2 * np.asarray(conv_b, np.float32).T)

    shared = dict(wpT=wpT, affT0=affT0, wpk=wpk, ab0=ab0, abr=abr,
                  cT0=cT0, gcb0=gcb0, gcbr=gcbr)
    in_maps = []
    for c in range(N_CORES):
        m = dict(shared)
        m["x"] = np.ascontiguousarray(x[:, :, c * SHARD:(c + 1) * SHARD])
        in_maps.append(m)
    return in_maps


def kernel(trace=False, **inputs):
    global _COMPILED
    if _COMPILED is None:
        _COMPILED = _build()
    nc = _COMPILED
    in_maps = _prep_inputs(**inputs)
    res = run_bass_kernel_spmd(nc, in_maps, core_ids=list(range(N_CORES)),
                               trace=trace)
    parts = [res.results[c]["y"] for c in range(N_CORES)]
    out = np.concatenate(parts, axis=2).reshape(B, HID, H, W)
    if trace:
        kernel.last_result = res
    return out

